# revision 38
# baseline (speedup 1.0000x reference)
"""Trainium2 Bass kernel for an attention block (MHSA with relative position
bias + 2x LayerNorm + FFN), sharded over 8 NeuronCores.

Sharding: tensor-parallel over heads for attention (core c owns head c, both
batch elements). Heads are exchanged with a bf16 AllToAll (per-head outputs,
already normalized, transposed [feat, tok]) so each core then computes the
out-projection, residual+LN1, FFN and LN2 for its own 512-token shard
locally. The host reassembles the full [2, 2048, 512] output.

Key layout choices:
  - All matmul inputs are bf16 (PSUM accumulation stays fp32); fp32r and
    bf16 stream at the same 1 row/cycle, but bf16 halves DMA/SBUF traffic
    and enables the DVE 2x 16-bit mode on the softmax multiply.
  - Scores are computed transposed (S^T[j,i]) so softmax normalization comes
    for free from a ones-column appended to V; no max-subtraction is needed
    because |scores/8 + bias| is bounded by ~10.
  - exp runs directly from PSUM with the 1/sqrt(dh) factor as the ACT scale;
    the relative-position bias is applied multiplicatively afterwards as
    exp(bias/8), read from a contiguous "band" load of the (reversed,
    pre-exponentiated) 1D table with a negative free stride in the DVE op.
  - Softmax 1/sigma is broadcast across partitions with a tiny ones-column
    PE matmul (no DRAM bounce) and applied in [feat, tok] layout.
  - Phase C runs ffn2(ihalf 0) before the out-projection of ihalf 1 so the
    second AllToAll is fully hidden behind FFN compute.
"""
import os
import sys

for _p in ("/opt/trn_rl_repo", "/root/.axon_site/_ro/trn_rl_repo"):
    if os.path.isdir(_p) and _p not in sys.path:
        sys.path.insert(0, _p)

import numpy as np
import ml_dtypes

import concourse.bass as bass
import concourse.mybir as mybir
import concourse.tile as tile
from concourse import bacc
from concourse import bass_utils

F32 = mybir.dt.float32
F32R = mybir.dt.float32r
BF16 = mybir.dt.bfloat16
AF = mybir.ActivationFunctionType
ALU = mybir.AluOpType

N_CORES = 8
B, L, D, DFF = 2, 2048, 512, 2048
H, DH = 8, 64
L_MAX = 39000
LN_EPS = 1e-5
NT = B * L               # 4096 tokens
P = 128
IH = 1024                # i-half size inside one batch's 2048 queries
NJT = L // P             # 16 j tiles per batch
SCALE = 0.125            # 1/sqrt(dh)
VW = DH + 2              # v block width (ones col + pad)

_cached = {}


def _ln_stats(nc, scr, h_aps, out_aps, eps_ap):
    """LayerNorm stats + normalize over the free dim (512) for a group of
    [128, 512] tiles; emits hn = (h - mu) * rstd WITHOUT the gamma/beta
    affine (applied by the caller, or folded downstream)."""
    n = len(h_aps)
    sq = [scr.tile([P, D], F32, tag=f"ln_sq{k}", name=f"lnsq{k}")
          for k in range(n)]
    ssum = [scr.tile([P, 1], F32, tag=f"ln_s1{k}", name=f"lns1{k}")
            for k in range(n)]
    msum = [scr.tile([P, 1], F32, tag=f"ln_s2{k}", name=f"lns2{k}")
            for k in range(n)]
    mu = [scr.tile([P, 1], F32, tag=f"ln_s3{k}", name=f"lns3{k}")
          for k in range(n)]
    var = [scr.tile([P, 1], F32, tag=f"ln_s5{k}", name=f"lns5{k}")
           for k in range(n)]
    std = [scr.tile([P, 1], F32, tag=f"ln_s6{k}", name=f"lns6{k}")
           for k in range(n)]
    rstd = [scr.tile([P, 1], F32, tag=f"ln_s7{k}", name=f"lns7{k}")
            for k in range(n)]
    nmr = [scr.tile([P, 1], F32, tag=f"ln_s8{k}", name=f"lns8{k}")
           for k in range(n)]

    # DVE-heavy: only the Sqrt crosses to ACT (2 engine hops total)
    for k in range(n):
        nc.vector.scalar_tensor_tensor(out=sq[k][:], in0=h_aps[k],
                                       scalar=1.0, in1=h_aps[k],
                                       op0=ALU.mult, op1=ALU.mult,
                                       accum_out=ssum[k][:])
    for k in range(n):
        nc.vector.tensor_reduce(out=msum[k][:], in_=h_aps[k],
                                axis=mybir.AxisListType.X, op=ALU.add)
    for k in range(n):
        nc.vector.tensor_scalar_mul(mu[k][:], msum[k][:], 1.0 / D)
        nc.vector.tensor_scalar_mul(var[k][:], ssum[k][:], 1.0 / D)
    for k in range(n):
        nc.vector.tensor_mul(msum[k][:], mu[k][:], mu[k][:])
        nc.vector.tensor_sub(var[k][:], var[k][:], msum[k][:])
    for k in range(n):
        nc.scalar.activation(std[k][:], var[k][:], AF.Sqrt, bias=eps_ap)
    for k in range(n):
        nc.vector.reciprocal(rstd[k][:], std[k][:])
        nc.vector.scalar_tensor_tensor(out=nmr[k][:], in0=mu[k][:],
                                       scalar=-1.0, in1=rstd[k][:],
                                       op0=ALU.mult, op1=ALU.mult)
    for k in range(n):
        nc.vector.tensor_scalar(out=out_aps[k], in0=h_aps[k],
                                scalar1=rstd[k][:], scalar2=nmr[k][:],
                                op0=ALU.mult, op1=ALU.add)


def build():
    nc = bacc.Bacc("TRN2", target_bir_lowering=False, debug=False,
                   num_devices=N_CORES)

    # ---- I/O ----
    xT = nc.dram_tensor("xT", [D, NT], BF16, kind="ExternalInput")
    xsh = nc.dram_tensor("xsh", [4 * P, D], F32, kind="ExternalInput")
    # wqk/wv are packed host-side as [128, c*out] so each loads in one DMA
    wqk = nc.dram_tensor("wqk", [P, 4 * P], BF16, kind="ExternalInput")
    wv = nc.dram_tensor("wv", [P, 4 * DH], BF16, kind="ExternalInput")
    wo = nc.dram_tensor("wo", [D, D], BF16, kind="ExternalInput")
    w1 = nc.dram_tensor("w1", [D, DFF], BF16, kind="ExternalInput")
    w2 = nc.dram_tensor("w2", [DFF, D], BF16, kind="ExternalInput")
    etrev = nc.dram_tensor("etrev", [4096], BF16, kind="ExternalInput")
    bqk = nc.dram_tensor("bqk", [2 * DH, 1], F32, kind="ExternalInput")
    bv = nc.dram_tensor("bv", [DH, 1], F32, kind="ExternalInput")
    b1c = nc.dram_tensor("b1c", [P, DFF // P], F32, kind="ExternalInput")
    g1c = nc.dram_tensor("g1c", [P, D // P], F32, kind="ExternalInput")
    b2 = nc.dram_tensor("b2", [D], F32, kind="ExternalInput")
    g1 = nc.dram_tensor("g1", [D], F32, kind="ExternalInput")
    g2 = nc.dram_tensor("g2", [D], F32, kind="ExternalInput")
    be2 = nc.dram_tensor("be2", [D], F32, kind="ExternalInput")
    out_sh = nc.dram_tensor("out_sh", [4 * P, D], F32, kind="ExternalOutput")

    with tile.TileContext(nc) as tc:
        with tc.tile_pool(name="persist", bufs=1) as pers, \
             tc.tile_pool(name="phC_w", bufs=1) as pCw, \
             tc.tile_pool(name="dram", bufs=1, space="DRAM") as dram:

            # ---------- persistent SBUF (small) ----------
            ident32 = pers.tile([P, P], F32)
            identr = pers.tile([P, P], F32R)
            bqk_sb = pers.tile([2 * DH, 1], F32)
            bv_sb = pers.tile([DH, 1], F32)
            wo_sb = [pers.tile([P, D], BF16, name=f"wo{c}") for c in range(4)]
            eps_sb = pers.tile([P, 1], F32)
            nc.vector.memset(eps_sb[:], LN_EPS)
            ones_sb = pers.tile([P, 1], F32)
            nc.vector.memset(ones_sb[:], 1.0)
            ones_bf = pers.tile([1, DH], BF16)
            nc.vector.memset(ones_bf[:], 1.0)
            warm_sb = pers.tile([1, 1], F32)
            oa_sb = [[pers.tile([P, 2 * P], BF16, name=f"oa_{ih}_{c}")
                      for c in range(4)] for ih in range(2)]
            hh0_sb = [pers.tile([P, D], F32, name=f"hh0_{b_}")
                      for b_ in range(B)]
            x0_sb = [pers.tile([P, D], F32, name=f"x0_{b_}")
                     for b_ in range(B)]

            # a2a buffers: per ihalf, [8 pieces x 64 feat, b0(128)|b1(128)]
            a2a_in = [dram.tile([N_CORES * DH, 2 * P], BF16, name=f"a2ai{i}")
                      for i in range(2)]
            a2a_out = [dram.tile([N_CORES * DH, 2 * P], BF16, name=f"a2ao{i}")
                       for i in range(2)]
            rr_dram = [dram.tile([1, IH], F32, name=f"rr{b_}")
                       for b_ in range(B)]

            # ================= attention (phases A+B) =================
            with tc.tile_pool(name="attn_sb", bufs=1) as patt:
                qkT = patt.tile([P, NT], BF16)       # [q(0:64)|k(64:128), tok]
                kT_sb = patt.tile([DH, NT], BF16)    # k^T re-based to part 0
                v_aug = patt.tile([P, 32 * VW], BF16)
                nc.vector.memset(v_aug[:], 1.0)      # ones cols pre-set
                outT_sb = [patt.tile([DH, L], BF16, name=f"outT{b_}")
                           for b_ in range(B)]

                # ---------- phase A: qkv projections ----------
                with tc.tile_pool(name="phA", bufs=1) as pA, \
                     tc.tile_pool(name="psA", bufs=2, space="PSUM") as psA:
                    qeng = [nc.sync, nc.scalar, nc.gpsimd, nc.sync]
                    xhs = {}

                    def load_xh(hf):
                        xh = [pA.tile([P, L], BF16, tag=f"xh{c}",
                                      name=f"xh_{hf}_{c}", bufs=2)
                              for c in range(4)]
                        xhs[hf] = xh
                        for half in range(2):
                            cs = slice(half * IH, (half + 1) * IH)
                            for c in range(4):
                                qeng[c].dma_start(
                                    xh[c][:, cs],
                                    xT.ap()[c * P:(c + 1) * P,
                                            hf * L + half * IH:
                                            hf * L + (half + 1) * IH])

                    # x chunk loads go out first — everything else on the
                    # DMA queues would delay the first matmul
                    load_xh(0)
                    wqk_sb = pA.tile([P, 4 * P], BF16, name="wqk")
                    wv_sb = pA.tile([P, 4 * DH], BF16, name="wv")
                    nc.sync.dma_start(wqk_sb[:], wqk.ap())
                    nc.gpsimd.dma_start(wv_sb[:], wv.ap())
                    nc.sync.dma_start(bqk_sb[:], bqk.ap())
                    nc.sync.dma_start(bv_sb[:], bv.ap())
                    load_xh(1)
                    for c in range(4):
                        nc.scalar.dma_start(wo_sb[c][:],
                                            wo.ap()[c * P:(c + 1) * P, :])

                    from concourse.masks import make_identity
                    make_identity(nc, ident32[:])
                    nc.scalar.copy(identr[:], ident32[:])

                    vT_sb = pA.tile([DH, NT], F32R)
                    for hf in range(2):
                        xh = xhs[hf]
                        for t4 in range(4):
                            t = hf * 4 + t4
                            ps = psA.tile([P, 512], F32, tag="qk_ps")
                            for c in range(4):
                                nc.tensor.matmul(
                                    ps[:], wqk_sb[:, c * P:(c + 1) * P],
                                    xh[c][:, t4 * 512:(t4 + 1) * 512],
                                    start=(c == 0), stop=(c == 3))
                            nc.vector.tensor_scalar_add(
                                qkT[:, t * 512:(t + 1) * 512], ps[:],
                                bqk_sb[:])
                            nc.sync.dma_start(kT_sb[:, t * 512:(t + 1) * 512],
                                              qkT[DH:, t * 512:(t + 1) * 512])
                            psv = psA.tile([DH, 512], F32, tag="v_ps")
                            for c in range(4):
                                nc.tensor.matmul(
                                    psv[:], wv_sb[:, c * DH:(c + 1) * DH],
                                    xh[c][:, t4 * 512:(t4 + 1) * 512],
                                    start=(c == 0), stop=(c == 3))
                            nc.vector.tensor_scalar_add(
                                vT_sb[:, t * 512:(t + 1) * 512], psv[:],
                                bv_sb[:])
                    # transpose v^T -> v_aug natural [tok, dh] (bf16, with
                    # the pre-set ones cols at 64:66 of each block)
                    for t in range(32):
                        ps = psA.tile([P, DH], F32R, tag="vt_ps")
                        nc.tensor.transpose(ps[:], vT_sb[:, t * P:(t + 1) * P],
                                            identr[:DH, :DH])
                        nc.scalar.copy(
                            v_aug[:, t * VW:t * VW + DH], ps[:])

                # prefetch FFN weights now that phase A freed SBUF;
                # keep their issue cost off the (busy) ACT queue
                w1_sb = [pCw.tile([P, DFF], BF16, name=f"w1_{c}")
                         for c in range(4)]
                for c in range(4):
                    nc.scalar.dma_start(w1_sb[c][:],
                                        w1.ap()[c * P:(c + 1) * P, :])
                w2_sb = [pCw.tile([P, D], BF16, name=f"w2_{q}")
                         for q in range(16)]
                for q in range(16):
                    eng = nc.gpsimd if q % 2 == 0 else nc.scalar
                    eng.dma_start(w2_sb[q][:], w2.ap()[q * P:(q + 1) * P, :])
                b1_sb = pCw.tile([P, DFF // P], F32)
                nc.scalar.dma_start(b1_sb[:], b1c.ap())
                g1c_sb = pCw.tile([P, D // P], F32)
                nc.scalar.dma_start(g1c_sb[:], g1c.ap())
                reps = {}
                for nm, t in (("b2", b2), ("g1", g1),
                              ("g2", g2), ("be2", be2)):
                    r = pCw.tile([P, D], F32, name=f"rep_{nm}")
                    nc.gpsimd.dma_start(
                        r[:], t.ap().unsqueeze(0).broadcast_to([P, D]))
                    reps[nm] = r

                # ---------- phase B: attention ----------
                with tc.tile_pool(name="phB_w", bufs=4) as pW, \
                     tc.tile_pool(name="phB_p", bufs=3) as pP, \
                     tc.tile_pool(name="phB_r", bufs=2) as pR, \
                     tc.tile_pool(name="psB_s", bufs=2, space="PSUM") as psS, \
                     tc.tile_pool(name="psB_o", bufs=2, space="PSUM") as psO:
                    for b_ in range(B):
                        nc.sync.dma_start(
                            x0_sb[b_][:],
                            xsh.ap()[b_ * 2 * P:(b_ * 2 + 1) * P, :])
                    for ihalf in range(2):
                        i0g = ihalf * IH
                        oT = [psO.tile([VW, IH], F32, tag="outT_ps",
                                       name=f"oT_{ihalf}_{b_}")
                              for b_ in range(B)]
                        for jt in range(NJT):
                            # band: W[j', m] = etrev[q0 + j' + m]
                            q0 = jt * P + IH * (1 - ihalf)
                            wband = pW.tile([P, IH], BF16, tag="wband")
                            nc.sync.dma_start(
                                wband[:],
                                bass.AP(etrev, q0, [[1, P], [1, IH]]))
                            wrev = wband[:, IH - 1::-1]  # exp(bias/8)^T tile
                            # both batches' scores go to the PE first so the
                            # exp->mult chain of b0 hides under b1's scores
                            pts = []
                            for b_ in range(B):
                                tb = b_ * L
                                sps = psS.tile([P, IH], F32, tag="s_ps")
                                for n2 in range(2):
                                    nc.tensor.matmul(
                                        sps[:, n2 * 512:(n2 + 1) * 512],
                                        kT_sb[:, tb + jt * P:
                                              tb + (jt + 1) * P],
                                        qkT[:DH, tb + i0g + n2 * 512:
                                            tb + i0g + (n2 + 1) * 512],
                                        start=True, stop=True)
                                pt0 = pP.tile([P, IH], BF16, tag="pt0")
                                nc.scalar.activation(pt0[:], sps[:], AF.Exp,
                                                     scale=SCALE)
                                pt = pP.tile([P, IH], BF16, tag="pt")
                                nc.vector.tensor_mul(pt[:], pt0[:], wrev)
                                pts.append(pt)
                            for b_ in range(B):
                                vs = v_aug[:, (b_ * NJT + jt) * VW:
                                           (b_ * NJT + jt) * VW + DH + 2]
                                for n2 in range(2):
                                    nc.tensor.matmul(
                                        oT[b_][:, n2 * 512:(n2 + 1) * 512],
                                        vs,
                                        pts[b_][:, n2 * 512:(n2 + 1) * 512],
                                        start=(jt == 0),
                                        stop=(jt == NJT - 1))
                        if ihalf == 1:
                            # hoisted: out-proj + residual for ihalf 0 fills
                            # the PE while the evict/a2a chain runs
                            for b_ in range(B):
                                pop0 = psS.tile([P, IH], F32, tag="s_ps",
                                                name=f"pop0_{b_}")
                                for c in range(4):
                                    nc.tensor.matmul(
                                        pop0[:, :D],
                                        oa_sb[0][c][:, b_ * P:(b_ + 1) * P],
                                        wo_sb[c][:], start=(c == 0),
                                        stop=(c == 3))
                                nc.vector.scalar_tensor_tensor(
                                    out=hh0_sb[b_][:], in0=pop0[:, :D],
                                    scalar=1.0, in1=x0_sb[b_][:],
                                    op0=ALU.mult, op1=ALU.add)
                        # evict: normalize by 1/sigma. Broadcast sigma across
                        # partitions with a ones-column PE matmul, reciprocal
                        # into SBUF, then a single-PSUM-operand multiply;
                        # ship each batch as soon as it is normalized.
                        for b_ in range(B):
                            sl = slice(i0g, i0g + IH)
                            if True:
                                # PE ones-column broadcast: fastest path to
                                # the ship that gates the last AllToAll
                                sgbf = pR.tile([1, IH], BF16, tag="sgbf")
                                nc.scalar.copy(sgbf[:], oT[b_][DH:DH + 1, :])
                                bc = psS.tile([P, IH], F32, tag="s_ps",
                                              name=f"bc_{b_}")
                                for n2 in range(2):
                                    nc.tensor.matmul(
                                        bc[:DH, n2 * 512:(n2 + 1) * 512],
                                        ones_bf[:],
                                        sgbf[:, n2 * 512:(n2 + 1) * 512],
                                        start=True, stop=True)
                                rdiv = pR.tile([DH, IH], F32, tag="rdiv")
                                rscr = pR.tile([DH, IH], F32, tag="rscr")
                                nc.vector.reciprocal_approx_accurate(
                                    rdiv[:], bc[:DH, :], rscr[:])
                            nc.vector.tensor_mul(outT_sb[b_][:, sl],
                                                 oT[b_][:DH, :], rdiv[:])
                            eng = nc.sync if b_ == 0 else nc.scalar
                            eng.dma_start(
                                a2a_in[ihalf][:].rearrange(
                                    "(cd p) (b2 col) -> p cd b2 col",
                                    cd=N_CORES, b2=2
                                )[:, :, b_, :],
                                outT_sb[b_][:, i0g:i0g + IH])
                        if ihalf == 1:
                            # prewarm ACT Sqrt table so LN1 doesn't pay the
                            # table reload on the post-attention critical path
                            nc.scalar.activation(warm_sb[:], eps_sb[:1, :1],
                                                 AF.Sqrt, bias=0.0)
                        nc.gpsimd.collective_compute(
                            "AllToAll", ALU.bypass,
                            replica_groups=[list(range(N_CORES))],
                            ins=[a2a_in[ihalf][:].opt()],
                            outs=[a2a_out[ihalf][:].opt()])
                        for c in range(4):
                            nc.gpsimd.dma_start(
                                oa_sb[ihalf][c][:],
                                a2a_out[ihalf][c * P:(c + 1) * P, :])

            # ---------- phase C: out-proj + residual + LN1 + FFN + LN2 ----
            with tc.tile_pool(name="phC", bufs=2) as pC, \
                 tc.tile_pool(name="phC_scr", bufs=2) as scr, \
                 tc.tile_pool(name="psC", bufs=2, space="PSUM") as psC, \
                 tc.tile_pool(name="psC2", bufs=2, space="PSUM") as psC2, \
                 tc.tile_pool(name="psC3", bufs=2, space="PSUM") as psC3:
                hns = {}
                hg2s = {}
                hgTs = {}
                f1Ts = {}

                def prep(ih):
                    # hn is F32R (fp32 bits) so the transpose runs at 1.5
                    # cycles/row; it is pre-affine — gamma1 is applied via
                    # the ACT copy scale (transposed) and via hg2 for the
                    # residual; beta1 is folded into b1/b2 host-side
                    hn = [pC.tile([P, D], F32R, tag="hn", bufs=5,
                                  name=f"hn_{ih}_{b_}") for b_ in range(B)]
                    hns[ih] = hn
                    if ih == 0:
                        hhs = hh0_sb
                    else:
                        hhs = []
                        for b_ in range(B):
                            ci = b_ * 2 + ih
                            pop = psC3.tile([P, D], F32, tag="po_ps")
                            for c in range(4):
                                nc.tensor.matmul(
                                    pop[:],
                                    oa_sb[ih][c][:, b_ * P:(b_ + 1) * P],
                                    wo_sb[c][:], start=(c == 0),
                                    stop=(c == 3))
                            x_sb = pC.tile([P, D], F32, tag="x_sb")
                            nc.sync.dma_start(
                                x_sb[:], xsh.ap()[ci * P:(ci + 1) * P, :])
                            hh = pC.tile([P, D], F32, tag="hh", bufs=4,
                                         name=f"hh_{ih}_{b_}")
                            nc.vector.scalar_tensor_tensor(
                                out=hh[:], in0=pop[:], scalar=1.0,
                                in1=x_sb[:], op0=ALU.mult, op1=ALU.add)
                            hhs.append(hh)
                    _ln_stats(nc, scr, [t[:] for t in hhs],
                              [hn[b_][:] for b_ in range(B)], eps_sb[:])

                def transposes(ih):
                    hgT = pC.tile([P, 4 * 256], BF16, tag="hgT", bufs=2,
                                  name=f"hgT_{ih}")
                    hgTs[ih] = hgT
                    for b_ in range(B):
                        for c in range(4):
                            tps = psC.tile([P, P], F32R, tag="tr_ps")
                            nc.tensor.transpose(
                                tps[:], hns[ih][b_][:, c * P:(c + 1) * P],
                                identr[:])
                            # gamma1 applied as a per-partition scale in the
                            # transposed layout
                            nc.scalar.activation(
                                hgT[:, c * 256 + b_ * P:
                                    c * 256 + (b_ + 1) * P],
                                tps[:], AF.Identity,
                                scale=g1c_sb[:, c:c + 1])
                    # residual copy hg2 = hn * gamma1 (off the critical path)
                    hg2 = [pC.tile([P, D], F32, tag="hg2", bufs=4,
                                   name=f"hg2_{ih}_{b_}") for b_ in range(B)]
                    hg2s[ih] = hg2
                    for b_ in range(B):
                        nc.vector.tensor_mul(hg2[b_][:], hns[ih][b_][:],
                                             reps["g1"][:])

                def ffn1(ih):
                    f1T = pC.tile([P, 16 * 256], BF16, tag="f1T", bufs=2,
                                  name=f"f1T_{ih}")
                    f1Ts[ih] = f1T
                    for t in range(16):
                        fps = psC.tile([P, 256], F32, tag="f1_ps")
                        for c in range(4):
                            nc.tensor.matmul(
                                fps[:], w1_sb[c][:, t * P:(t + 1) * P],
                                hgTs[ih][:, c * 256:(c + 1) * 256],
                                start=(c == 0), stop=(c == 3))
                        # relu+bias on DVE (keeps the ACT Sqrt table warm)
                        nc.vector.tensor_scalar(
                            out=f1T[:, t * 256:(t + 1) * 256], in0=fps[:],
                            scalar1=b1_sb[:, t:t + 1], scalar2=0.0,
                            op0=ALU.add, op1=ALU.max)

                def ffn2_out(ih):
                    for b_ in range(B):
                        ops = psC2.tile([P, D], F32, tag="f2_ps")
                        for q in range(16):
                            nc.tensor.matmul(
                                ops[:],
                                f1Ts[ih][:, q * 256 + b_ * P:
                                         q * 256 + (b_ + 1) * P],
                                w2_sb[q][:], start=(q == 0), stop=(q == 15))
                        zz = pC.tile([P, D], F32, tag="zz", bufs=4,
                                     name=f"zz_{ih}_{b_}")
                        nc.vector.tensor_add(zz[:], ops[:], hg2s[ih][b_][:])
                        nc.vector.tensor_add(zz[:], zz[:], reps["b2"][:])
                        hn2 = pC.tile([P, D], F32, tag="hn2", bufs=4,
                                      name=f"hn2_{ih}_{b_}")
                        _ln_stats(nc, scr, [zz[:]], [hn2[:]], eps_sb[:])
                        yy = pC.tile([P, D], F32, tag="yy", bufs=4,
                                     name=f"yy_{ih}_{b_}")
                        nc.vector.tensor_mul(yy[:], hn2[:], reps["g2"][:])
                        nc.vector.tensor_add(yy[:], yy[:], reps["be2"][:])
                        ci = b_ * 2 + ih
                        nc.sync.dma_start(out_sh.ap()[ci * P:(ci + 1) * P, :],
                                          yy[:])

                prep(0)
                transposes(0)
                ffn1(0)
                ffn2_out(0)
                prep(1)
                transposes(1)
                ffn1(1)
                ffn2_out(1)
    nc.compile()
    return nc


def _prep_inputs(x, Wqkv, bqkv, Wo, bo, pos_bias, W1, b1, W2, b2,
                 gamma1, beta1, gamma2, beta2):
    x_flat = np.ascontiguousarray(x.reshape(NT, D), dtype=np.float32)
    xT = np.ascontiguousarray(x_flat.T).astype(ml_dtypes.bfloat16)
    wo_full = np.asarray(Wo, np.float32).astype(ml_dtypes.bfloat16)
    w1_bf = np.asarray(W1, np.float32).astype(ml_dtypes.bfloat16)
    w2_bf = np.asarray(W2, np.float32).astype(ml_dtypes.bfloat16)
    in_maps = []
    for c in range(N_CORES):
        h = c
        base = h * 3 * DH
        wqk_h = np.ascontiguousarray(
            np.concatenate([Wqkv[:, base:base + DH],
                            Wqkv[:, base + DH:base + 2 * DH]], axis=1),
            dtype=np.float32).astype(ml_dtypes.bfloat16)
        # pack [512, out] -> [128, 4*out] so each weight loads in one DMA
        wqk_h = np.ascontiguousarray(
            wqk_h.reshape(4, P, 2 * DH).transpose(1, 0, 2).reshape(P, 4 * P))
        wv_h = np.ascontiguousarray(
            Wqkv[:, base + 2 * DH:base + 3 * DH],
            dtype=np.float32).astype(ml_dtypes.bfloat16)
        wv_h = np.ascontiguousarray(
            wv_h.reshape(4, P, DH).transpose(1, 0, 2).reshape(P, 4 * DH))
        bqk_h = np.concatenate([bqkv[base:base + DH],
                                bqkv[base + DH:base + 2 * DH]]).reshape(-1, 1)
        bv_h = bqkv[base + 2 * DH:base + 3 * DH].reshape(-1, 1)
        tbl = pos_bias[L_MAX - 1 - (L - 1):L_MAX - 1 + L, h].astype(np.float64)
        etrev = np.exp(tbl * SCALE)[::-1].astype(ml_dtypes.bfloat16)
        etrev = np.concatenate([etrev, np.ones(1, ml_dtypes.bfloat16)])
        # bo folded into the residual copies of x; beta1 folded into b1/b2
        xsh = np.empty((4 * P, D), np.float32)
        for ci, (b_, ihalf) in enumerate([(0, 0), (0, 1), (1, 0), (1, 1)]):
            r0 = b_ * L + ihalf * IH + c * P
            xsh[ci * P:(ci + 1) * P] = x_flat[r0:r0 + P] + bo[None, :]
        b1f = (np.asarray(b1, np.float64)
               + np.asarray(beta1, np.float64) @ np.asarray(W1, np.float64)
               ).astype(np.float32)
        b2f = (np.asarray(b2, np.float64)
               + np.asarray(beta1, np.float64)).astype(np.float32)
        in_maps.append({
            "xT": xT, "xsh": xsh, "wqk": wqk_h, "wv": wv_h, "wo": wo_full,
            "w1": w1_bf, "w2": w2_bf,
            "etrev": etrev,
            "bqk": np.asarray(bqk_h, np.float32),
            "bv": np.asarray(bv_h, np.float32),
            "b1c": np.ascontiguousarray(b1f.reshape(DFF // P, P).T),
            "g1c": np.ascontiguousarray(
                np.asarray(gamma1, np.float32).reshape(D // P, P).T),
            "b2": b2f,
            "g1": np.asarray(gamma1, np.float32),
            "g2": np.asarray(gamma2, np.float32),
            "be2": np.asarray(beta2, np.float32),
        })
    return in_maps


def kernel(**inputs):
    if "nc" not in _cached:
        _cached["nc"] = build()
    nc = _cached["nc"]
    in_maps = _prep_inputs(**{k: np.asarray(v) for k, v in inputs.items()})
    res = bass_utils.run_bass_kernel_spmd(
        nc, in_maps, core_ids=list(range(N_CORES)),
        **_cached.get("run_kwargs", {}))
    _cached["last_result"] = res
    out = np.empty((NT, D), np.float32)
    for c in range(N_CORES):
        sh = res.results[c]["out_sh"]
        for ci, (b_, ihalf) in enumerate([(0, 0), (0, 1), (1, 0), (1, 1)]):
            r0 = b_ * L + ihalf * IH + c * P
            out[r0:r0 + P] = sh[ci * P:(ci + 1) * P]
    return out.reshape(B, L, D)
